# revision 1
# baseline (speedup 1.0000x reference)
"""Multi-head causal attention (B=2, S=2048, H=16, DH=64, D=1024) on 8 TRN2 cores.

Sharding: Megatron tensor-parallel over heads — core c owns heads {2c, 2c+1}:
  * column-slices of Wq/Wk/Wv (128 cols each) + bias slices,
  * row-slice of Wo (128 rows),
  * full hidden_states (pre-transposed on host to [D, B*S]).
Each core computes a partial output (its 2 heads through Wo rows); host sums
the 8 partials (row-parallel unshard) — bo is added on device by core 0.

Device dataflow per core (all matmuls in float32r — full-rate fp32 with
15-bit mantissa — contract: every matmul operand must be produced by a
"rounding" instruction or DMA-ed from a float32r DRAM tensor):
  A) QKV: qT/kT/vT [128, 4096] = W_slice.T @ hiddenT  (contraction over D in
     8 chunks of 128; biases are per-partition adds during PSUM evacuation).
     vT is PE-transposed into V_aug [tok128, chunk, head, 65] with a ones
     column (65th) so the AV matmul also emits softmax denominators.
  B) Attention per (batch b, 256-query block qi), both heads:
     scoresT[kv,q] = kT_slice.T @ qT_slice  (K=64, N=256, two heads packed on
     PE row-groups 0-63/64-127), exp via ACT (scale=1/8) straight off PSUM,
     causal diag masked by a triangular multiply (even chunk) / restricting
     the AV matmul columns (odd chunk), then ctxT_aug[65, q] += V_aug.T@expT.
     Softmax denominators (row 64) are reciprocal-ed and broadcast across
     64 partitions with a K=1 ones matmul, then ctxT normalized via DVE.
  C) Output proj: outT[n,tok] = Wo_slice_chunk.T @ ctxT (+ bo via K=1 ones
     matmul, nonzero on core 0 only), DMA-ed to DRAM straight from PSUM.
"""
import os
import sys

sys.path.insert(0, "/opt/trn_rl_repo")

from contextlib import ExitStack

import numpy as np

import concourse.bass as bass
import concourse.mybir as mybir
import concourse.tile as tile
from concourse import bacc
from concourse.bass_utils import run_bass_kernel_spmd

F32 = mybir.dt.float32
F32R = mybir.dt.float32r
F16 = mybir.dt.float16
MM_DT = F32R if os.environ.get("KERNEL_F32R") == "1" else F16
MM_NP = np.float32 if os.environ.get("KERNEL_F32R") == "1" else np.float16

B, S, H, DH = 2, 2048, 16, 64
D = H * DH            # 1024
T = B * S             # 4096 tokens
NCORES = 8
HPC = H // NCORES     # 2 heads per core
KC = D // 128         # 8 contraction chunks
NG = T // 512         # 8 token groups for QKV
NQB = S // 256        # 8 query blocks per batch
NKV = T // 128        # 32 kv chunks of 128 tokens
EXPFN = mybir.ActivationFunctionType.Exp


def _body(nc, tc, ctx, t_in, t_out, t_out_dbg=None):
    xt, wq, wk, wv, wo, bq, bk, bv, bo, tri, ident, vones = t_in
    po = t_out

    const = ctx.enter_context(tc.tile_pool(name="const", bufs=1))
    big = ctx.enter_context(tc.tile_pool(name="big", bufs=1))
    xtp = ctx.enter_context(tc.tile_pool(name="xtp", bufs=3))
    vtp = ctx.enter_context(tc.tile_pool(name="vtp", bufs=2))
    ep = ctx.enter_context(tc.tile_pool(name="ep", bufs=6))
    rp = ctx.enter_context(tc.tile_pool(name="rp", bufs=2))
    rbp = ctx.enter_context(tc.tile_pool(name="rbp", bufs=2))

    psS = ctx.enter_context(tc.tile_pool(name="psS", bufs=3, space="PSUM"))
    psC = ctx.enter_context(tc.tile_pool(name="psC", bufs=1, space="PSUM"))

    # ---- constants / weights in SBUF
    wq_s = const.tile([128, KC, 128], MM_DT, tag="wq")
    wk_s = const.tile([128, KC, 128], MM_DT, tag="wk")
    wv_s = const.tile([128, KC, 128], MM_DT, tag="wv")
    wo_s = const.tile([128, D], MM_DT, tag="wo")
    bq_s = const.tile([128, 1], F32, tag="bq")
    bk_s = const.tile([128, 1], F32, tag="bk")
    bv_s = const.tile([128, 1], F32, tag="bv")
    bo_s = const.tile([128, KC], F32, tag="bo")
    tri_s = const.tile([128, 128], MM_DT, tag="tri")
    id_s = const.tile([128, 128], MM_DT, tag="ident")
    nc.sync.dma_start(wq_s[:], wq[:])
    nc.sync.dma_start(wk_s[:], wk[:])
    nc.sync.dma_start(wv_s[:], wv[:])
    nc.sync.dma_start(bq_s[:], bq[:])
    nc.sync.dma_start(bk_s[:], bk[:])
    nc.sync.dma_start(bv_s[:], bv[:])
    nc.gpsimd.dma_start(wo_s[:], wo[:])
    nc.gpsimd.dma_start(bo_s[:], bo[:])
    nc.gpsimd.dma_start(tri_s[:], tri[:])
    nc.gpsimd.dma_start(id_s[:], ident[:])

    qT = big.tile([128, T], MM_DT, tag="qT")
    kT = big.tile([128, T], MM_DT, tag="kT")
    vT = big.tile([128, T], MM_DT, tag="vT")
    ctxT = big.tile([128, T], MM_DT, tag="ctxT")
    vaug = big.tile([128, NKV, HPC, 65], MM_DT, tag="vaug")
    # ones column of V_aug (softmax denominator trick)
    nc.gpsimd.dma_start(vaug[:, :, :, 64].rearrange("p c h -> p (c h)"), vones[:])


    if t_out_dbg is not None:
        dbgp = ctx.enter_context(tc.tile_pool(name="dbgp", bufs=1))
        dbg_craw_s = dbgp.tile([128, T], F32, tag="craw")
        dbg_den_s = dbgp.tile([128, T], F32, tag="den")

    # ---- phase A: QKV projections + V transpose
    for g in range(NG):
        cols = slice(g * 512, (g + 1) * 512)
        xg = xtp.tile([128, KC, 512], MM_DT, tag="xt")
        nc.sync.dma_start(xg[:], xt[g])

        for (w_s, b_s, dst) in ((wq_s, bq_s, qT), (wk_s, bk_s, kT)):
            acc = psS.tile([128, 512], F32, tag="sp")
            for k in range(KC):
                nc.tensor.matmul(acc[:], w_s[:, k, :], xg[:, k, :],
                                 start=(k == 0), stop=(k == KC - 1))
            nc.vector.tensor_scalar_add(dst[:, cols], acc[:], b_s[:])

        acc = psS.tile([128, 512], F32, tag="sp")
        for k in range(KC):
            nc.tensor.matmul(acc[:], wv_s[:, k, :], xg[:, k, :],
                             start=(k == 0), stop=(k == KC - 1))
        nc.vector.tensor_scalar_add(vT[:, cols], acc[:], bv_s[:])

    # transpose vT into V_aug (natural [tok, dh] layout), batched after QKV
    for chunk in range(NKV):
        tp = psS.tile([128, 128], MM_DT, tag="sp")
        nc.tensor.transpose(tp[:], vT[:, chunk * 128:(chunk + 1) * 128],
                            id_s[:])
        nc.vector.tensor_copy(
            vaug[:, chunk, :, 0:64],
            tp[:].rearrange("p (h d) -> p h d", h=HPC))

    woc = wo_s[:].rearrange("p (c n) -> p c n", c=KC)
    osp = ctx.enter_context(tc.tile_pool(name="osp", bufs=6))

    # ---- phase B: attention, both heads, causal
    for qi in range(NQB):
        for b in range(B):
            qcols = slice(b * S + qi * 256, b * S + qi * 256 + 256)
            nch = 2 * (qi + 1)          # kv chunks of 128 for this q block
            ct0 = psC.tile([65, 256], F32, tag="ct0")
            ct1 = psC.tile([65, 256], F32, tag="ct1")
            cts = [ct0, ct1]
            nwaves = (nch + 1) // 2
            for w in range(nwaves):
                js = [j for j in (2 * w, 2 * w + 1) if j < nch]
                sp = psS.tile([128, 2 * HPC, 256], F32, tag="sp")
                for h in range(HPC):
                    for i, j in enumerate(js):
                        kcols = slice(b * S + j * 128, b * S + j * 128 + 128)
                        nc.tensor.matmul(
                            sp[:, 2 * h + i, :],
                            kT[h * 64:(h + 1) * 64, kcols],
                            qT[h * 64:(h + 1) * 64, qcols],
                            start=True, stop=True)
                e = ep.tile([128, 2 * HPC, 256], MM_DT, tag="e")
                nc.scalar.activation(e[:], sp[:], EXPFN, scale=0.125)
                for h in range(HPC):
                    for i, j in enumerate(js):
                        lhsT = vaug[:, b * (S // 128) + j, h, :]
                        first = (j == 0)
                        last = (j == nch - 1)
                        if j == nch - 2:  # even diag chunk: mask lower tri
                            nc.vector.tensor_mul(
                                e[:, 2 * h + i, 0:128],
                                e[:, 2 * h + i, 0:128], tri_s[:])
                            nc.tensor.matmul(cts[h][:, :], lhsT,
                                             e[:, 2 * h + i, :],
                                             start=first, stop=last)
                        elif j == nch - 1:  # odd diag: q first half all masked
                            nc.vector.tensor_mul(
                                e[:, 2 * h + i, 128:256],
                                e[:, 2 * h + i, 128:256], tri_s[:])
                            nc.tensor.matmul(cts[h][:, 128:256], lhsT,
                                             e[:, 2 * h + i, 128:256],
                                             start=first, stop=last)
                        else:
                            nc.tensor.matmul(cts[h][:, :], lhsT,
                                             e[:, 2 * h + i, :],
                                             start=first, stop=last)
            # normalize: ctxT[, q] = ct[0:64] * (1 / ct[64]) broadcast
            for h in range(HPC):
                sums = rp.tile([1, 256], F32, tag="sums")
                nc.vector.tensor_copy(sums[:], cts[h][64:65, :])
                r = rp.tile([1, 256], F32, tag="r")
                nc.vector.reciprocal_approx_fast(r[:], sums[:])
                rb = rbp.tile([64, 256], F32, tag="rb")
                nc.gpsimd.partition_broadcast(rb[:], r[:])
                nc.vector.tensor_mul(ctxT[h * 64:(h + 1) * 64, qcols],
                                     cts[h][0:64, :], rb[:])
                if t_out_dbg is not None:
                    nc.vector.tensor_copy(
                        dbg_craw_s[h * 64:(h + 1) * 64, qcols],
                        cts[h][0:64, :])
                    nc.vector.tensor_copy(
                        dbg_den_s[h * 64:(h + 1) * 64, qcols], rb[:])

            if qi % 2 == 1:
                # output projection for the completed 512-token block
                t = b * (S // 512) + qi // 2
                tcols = slice(t * 512, (t + 1) * 512)
                for c in range(KC):
                    op = psS.tile([128, 512], F32, tag="sp")
                    nc.tensor.matmul(op[:], woc[:, c, :], ctxT[:, tcols],
                                     start=True, stop=True)
                    ost = osp.tile([128, 512], F32, tag="ost")
                    if c % 2 == 0:
                        nc.vector.tensor_scalar_add(ost[:], op[:],
                                                    bo_s[:, c:c + 1])
                    else:
                        nc.scalar.activation(
                            ost[:], op[:],
                            mybir.ActivationFunctionType.Identity,
                            bias=bo_s[:, c:c + 1])
                    nc.sync.dma_start(po[c, :, tcols], ost[:])

    # ---- phase C: output projection (transposed out), bias, DMA from PSUM
    if t_out_dbg is not None:
        dq, dk, dc, dv, dcr, dde = t_out_dbg
        st = ctx.enter_context(tc.tile_pool(name="dbg", bufs=1))
        for src, dst in ((qT, dq), (kT, dk), (ctxT, dc)):
            tmp = st.tile([128, T], F32, tag="dbgt")
            nc.vector.tensor_copy(tmp[:], src[:])
            nc.sync.dma_start(dst[:], tmp[:])
        nc.sync.dma_start(dcr[:], dbg_craw_s[:])
        nc.sync.dma_start(dde[:], dbg_den_s[:])
        tmpv = st.tile([128, NKV * HPC * 65], F32, tag="dbgt")
        nc.vector.tensor_copy(
            tmpv[:], vaug[:].rearrange("p c h x -> p (c h x)"))
        nc.sync.dma_start(dv[:], tmpv[:])



_NC = None


def _build():
    global _NC
    if _NC is not None:
        return _NC
    nc = bacc.Bacc("TRN2", target_bir_lowering=False, debug=False,
                   num_devices=NCORES)
    t_in = [
        nc.dram_tensor("xt", [NG, 128, KC, 512], MM_DT, kind="ExternalInput").ap(),
        nc.dram_tensor("wq", [128, KC, 128], MM_DT, kind="ExternalInput").ap(),
        nc.dram_tensor("wk", [128, KC, 128], MM_DT, kind="ExternalInput").ap(),
        nc.dram_tensor("wv", [128, KC, 128], MM_DT, kind="ExternalInput").ap(),
        nc.dram_tensor("wo", [128, D], MM_DT, kind="ExternalInput").ap(),
        nc.dram_tensor("bq", [128, 1], F32, kind="ExternalInput").ap(),
        nc.dram_tensor("bk", [128, 1], F32, kind="ExternalInput").ap(),
        nc.dram_tensor("bv", [128, 1], F32, kind="ExternalInput").ap(),
        nc.dram_tensor("bo", [128, KC], F32, kind="ExternalInput").ap(),
        nc.dram_tensor("tri", [128, 128], MM_DT, kind="ExternalInput").ap(),
        nc.dram_tensor("ident", [128, 128], MM_DT, kind="ExternalInput").ap(),
        nc.dram_tensor("vones", [128, NKV * HPC], MM_DT,
                       kind="ExternalInput").ap(),
    ]
    po = nc.dram_tensor("po", [KC, 128, T], F32, kind="ExternalOutput").ap()
    t_out_dbg = None
    if os.environ.get("KERNEL_DEBUG_TAPS") == "1":
        t_out_dbg = [
            nc.dram_tensor("dbg_qT", [128, T], F32, kind="ExternalOutput").ap(),
            nc.dram_tensor("dbg_kT", [128, T], F32, kind="ExternalOutput").ap(),
            nc.dram_tensor("dbg_ctxT", [128, T], F32, kind="ExternalOutput").ap(),
            nc.dram_tensor("dbg_vaug", [128, NKV * HPC * 65], F32,
                           kind="ExternalOutput").ap(),
            nc.dram_tensor("dbg_craw", [128, T], F32, kind="ExternalOutput").ap(),
            nc.dram_tensor("dbg_den", [128, T], F32, kind="ExternalOutput").ap(),
        ]
    with tile.TileContext(nc) as tc, ExitStack() as ctx:
        _body(nc, tc, ctx, t_in, po, t_out_dbg)
    nc.compile()
    _NC = nc
    return nc


def _in_maps(hidden_states, Wq, bq, Wk, bk, Wv, bv, Wo, bo):
    hid = np.asarray(hidden_states, dtype=np.float32).reshape(T, D)
    hidT = hid.T.astype(MM_NP)                       # [D, T]
    xt = np.ascontiguousarray(
        hidT.reshape(KC, 128, NG, 512).transpose(2, 1, 0, 3))
    common = {
        "xt": xt,
        "tri": np.triu(np.ones((128, 128), MM_NP)),
        "ident": np.eye(128, dtype=MM_NP),
        "vones": np.ones((128, NKV * HPC), MM_NP),
    }
    maps = []
    for c in range(NCORES):
        cs = slice(c * 128, (c + 1) * 128)
        maps.append(dict(
            common,
            wq=np.ascontiguousarray(np.asarray(Wq)[:, cs].astype(MM_NP).reshape(KC, 128, 128).transpose(1, 0, 2)),
            wk=np.ascontiguousarray(np.asarray(Wk)[:, cs].astype(MM_NP).reshape(KC, 128, 128).transpose(1, 0, 2)),
            wv=np.ascontiguousarray(np.asarray(Wv)[:, cs].astype(MM_NP).reshape(KC, 128, 128).transpose(1, 0, 2)),
            wo=np.ascontiguousarray(np.asarray(Wo)[cs, :].astype(MM_NP)),
            bq=np.asarray(bq)[cs].reshape(128, 1).astype(np.float32),
            bk=np.asarray(bk)[cs].reshape(128, 1).astype(np.float32),
            bv=np.asarray(bv)[cs].reshape(128, 1).astype(np.float32),
            bo=(np.ascontiguousarray(
                    np.asarray(bo).astype(np.float32).reshape(KC, 128).T)
                if c == 0 else np.zeros((128, KC), np.float32)),
        ))
    return maps


def kernel(hidden_states, Wq, bq, Wk, bk, Wv, bv, Wo, bo):
    nc = _build()
    maps = _in_maps(hidden_states, Wq, bq, Wk, bk, Wv, bv, Wo, bo)
    res = run_bass_kernel_spmd(nc, maps, list(range(NCORES))).results
    acc = np.zeros((KC, 128, T), np.float64)
    for r in res:
        acc += r["po"]
    outT = acc.reshape(D, T)
    return outT.T.reshape(B, S, D).astype(np.float32)



# revision 35
# speedup vs baseline: 1.2997x; 1.2997x over previous
"""Multi-head causal attention (B=2, S=2048, H=16, DH=64, D=1024) on 8 TRN2 cores.

Sharding: Megatron tensor-parallel over heads - core c owns heads {2c, 2c+1}:
  * column-slices of Wq/Wk/Wv (128 cols each) + bias slices,
  * row-slice of Wo (128 rows),
  * full hidden_states (pre-transposed on host to [D, B*S]).
Each core computes a partial output (its 2 heads through Wo rows); host sums
the 8 partials and adds bo + bv@Wo (the v-bias commutes through softmax-
weighted averaging and the output projection).

Device dataflow per core (fp16 matmuls):
  A) QKV: qT/kT/vT [128, 4096] = W_slice.T @ hiddenT (contraction over D in
     8 chunks of 128; q/k biases are per-partition adds during PSUM
     evacuation, v bias is folded into the host-side output bias).
     vT is DMA-transposed (XBAR) into V_aug [tok128, chunk, head, 65] whose
     65th column is set to 1 (memset) so the AV matmul also emits softmax
     denominators.
  B) Attention per (batch b, 256-query block qi), both heads, waves of 2 kv
     chunks: scoresT[kv,q] = kT_h.T @ qT_h (K=64, N=256), exp via ACT
     (scale=1/8) straight off PSUM, causal diagonal handled by a triangular
     multiply (even chunk) / restricting scores+AV to the second 128 queries
     (odd chunk), then ctxT_aug[65, q] += V_aug.T @ expT. Denominators
     (row 64) are reciprocal-ed off PSUM and partition-broadcast (gpsimd),
     then ctxT normalized via DVE.
  C) Output proj: outT[n,tok] = Wo_slice_chunk.T @ ctxT, DMA-ed to DRAM
     straight from PSUM (f32), no bias on device.

The emission order software-pipelines everything: attention waves for block
w are interleaved with the previous wave's AV matmuls, and "filler" PE work
(QKV for later token groups, output projections of finished 512-token
blocks) is slotted between waves so the tensor engine never goes idle (idle
gaps reset the PE to its mid p-state: 1.2 GHz instead of 2.4 GHz).
"""
import os
import sys

sys.path.insert(0, "/opt/trn_rl_repo")

from contextlib import ExitStack

import numpy as np

import concourse.bass as bass
import concourse.mybir as mybir
import concourse.tile as tile
from concourse import bacc
from concourse.bass_utils import run_bass_kernel_spmd

F32 = mybir.dt.float32
F16 = mybir.dt.float16
MM_DT = F16
MM_NP = np.float16

B, S, H, DH = 2, 2048, 16, 64
D = H * DH            # 1024
T = B * S             # 4096 tokens
NCORES = 8
HPC = H // NCORES     # 2 heads per core
KC = D // 128         # 8 contraction chunks
NG = T // 512         # 8 token groups for QKV
NQB = S // 256        # 8 query blocks per batch
NKV = T // 128        # 32 kv chunks of 128 tokens
EXPFN = mybir.ActivationFunctionType.Exp


def _body(nc, tc, ctx, t_in, t_out, t_dbg=None):
    xt, wq, wk, wv, wo, bq, bk, tri2, ident, vones = t_in
    po = t_out

    const = ctx.enter_context(tc.tile_pool(name="const", bufs=1))
    big = ctx.enter_context(tc.tile_pool(name="big", bufs=1))
    ep = ctx.enter_context(tc.tile_pool(name="ep", bufs=4))
    rp = ctx.enter_context(tc.tile_pool(name="rp", bufs=4))
    rbp = ctx.enter_context(tc.tile_pool(name="rbp", bufs=4))

    psQ = ctx.enter_context(tc.tile_pool(name="psQ", bufs=2, space="PSUM"))
    psS = ctx.enter_context(tc.tile_pool(name="psS", bufs=2, space="PSUM"))
    psC = ctx.enter_context(tc.tile_pool(name="psC", bufs=1, space="PSUM"))

    # ---- constants / weights in SBUF (gpsimd ring, parallel with xt on sync)
    wq_s = const.tile([128, KC, 128], MM_DT, tag="wq")
    wk_s = const.tile([128, KC, 128], MM_DT, tag="wk")
    wv_s = const.tile([128, KC, 128], MM_DT, tag="wv")
    wo_s = const.tile([128, D], MM_DT, tag="wo")
    bq_s = const.tile([128, 1], F32, tag="bq")
    bk_s = const.tile([128, 1], F32, tag="bk")
    tri_s = const.tile([128, 2, 128], MM_DT, tag="tri")
    id_s = const.tile([128, 128], MM_DT, tag="ident")
    nc.sync.dma_start(wq_s[:], wq[:])
    nc.sync.dma_start(wk_s[:], wk[:])
    nc.sync.dma_start(wv_s[:], wv[:])
    nc.sync.dma_start(bq_s[:], bq[:])
    nc.sync.dma_start(bk_s[:], bk[:])
    nc.gpsimd.dma_start(id_s[:], ident[:])
    nc.gpsimd.dma_start(wo_s[:], wo[:])
    nc.gpsimd.dma_start(tri_s[:], tri2[:])

    qT = big.tile([128, T], MM_DT, tag="qT")
    kT = big.tile([128, T], MM_DT, tag="kT")
    vT = big.tile([128, T], MM_DT, tag="vT")
    ctxT = big.tile([128, T], MM_DT, tag="ctxT")
    vaug = big.tile([128, NKV, HPC, 65], MM_DT, tag="vaug")
    xall = big.tile([128, NG, KC, 512], MM_DT, tag="xall")
    # ones column of V_aug (softmax denominator trick)
    nc.gpsimd.dma_start(vaug[:, :, :, 64].rearrange("p c h -> p (c h)"),
                        vones[:])

    # stream the full transposed input in per-group chunks on the sync ring
    for g in range(NG):
        nc.sync.dma_start(xall[:, g], xt[g])

    woc = wo_s[:].rearrange("p (c n) -> p c n", c=KC)

    # ---------------- QKV projection emission (per token group) ----------
    def qkv_steps(g):
        """Yield filler steps (2 matmuls each) for token group g."""
        cols = slice(g * 512, (g + 1) * 512)
        for w_s, b_s, dst in ((wq_s, bq_s, qT), (wk_s, bk_s, kT),
                              (wv_s, None, vT)):
            # allocate the PSUM accumulator lazily (at emission time) so the
            # pool's WAR dependencies always point backward in the PE stream
            box = [None]

            def mk(k0, box=box, w_s=w_s, b_s=b_s, dst=dst, g=g, cols=cols):
                if k0 == 0:
                    box[0] = psQ.tile([128, 512], F32, tag="acc",
                                      name="acc")
                acc = box[0]
                for k in (k0, k0 + 1):
                    nc.tensor.matmul(acc[:], w_s[:, k, :], xall[:, g, k, :],
                                     start=(k == 0), stop=(k == KC - 1))
                if k0 + 2 == KC:
                    # evacuate PSUM (+bias for q/k) on the vector engine
                    if b_s is not None:
                        nc.vector.tensor_scalar_add(dst[:, cols], acc[:],
                                                    b_s[:])
                    else:
                        nc.vector.tensor_copy(dst[:, cols], acc[:])
                        # transpose v into V_aug via the PE
                        for c in range(4):
                            chunk = g * 4 + c
                            tp = psQ.tile([128, 128], MM_DT, tag="acc",
                                          name="tp")
                            nc.tensor.transpose(
                                tp[:], vT[:, chunk * 128:(chunk + 1) * 128],
                                id_s[:])
                            nc.vector.tensor_copy(
                                vaug[:, chunk, :, 0:64],
                                tp[:].rearrange("p (h d) -> p h d", h=HPC))

            for k0 in range(0, KC, 2):
                yield ("qkv", g, mk, k0)

    # ---------------- output projection emission (per 512-token block) ---
    osp = ctx.enter_context(tc.tile_pool(name="osp", bufs=4))
    evac_rr = [0]

    def outproj_steps(t):
        tcols = slice(t * 512, (t + 1) * 512)

        def mk(c, tcols=tcols, t=t):
            op = psQ.tile([128, 512], F32, tag="acc")
            nc.tensor.matmul(op[:], woc[:, c, :], ctxT[:, tcols],
                             start=True, stop=True)
            ost = osp.tile([128, 512], MM_DT, tag="ost")
            # alternate the PSUM evacuation between vector and scalar
            # (gpsimd cannot read PSUM)
            i = evac_rr[0] % 2
            evac_rr[0] += 1
            if i == 0:
                nc.vector.tensor_copy(ost[:], op[:])
            else:
                nc.scalar.activation(ost[:], op[:],
                                     mybir.ActivationFunctionType.Copy)
            nc.sync.dma_start(po[c, :, tcols], ost[:])

        for c in range(KC):
            yield ("op", t, mk, c)

    # ---------------- filler machinery ----------------------------------
    fillers = []          # list of (tag, ident, fn, arg)
    fpos = [0]

    def emit_filler(n):
        end = min(fpos[0] + n, len(fillers))
        for i in range(fpos[0], end):
            tag, ident, fn, arg = fillers[i]
            fn(arg)
        fpos[0] = end

    def drain_qkv_through(g):
        """Force-emit all queued QKV steps for groups <= g."""
        i = fpos[0]
        while i < len(fillers) and fillers[i][0] == "qkv" \
                and fillers[i][1] <= g:
            tag, ident, fn, arg = fillers[i]
            fn(arg)
            i += 1
        fpos[0] = i

    # groups 0,1 emitted up front (needed before any attention); the rest
    # become fillers
    for g in (0, 1):
        for _, _, fn, arg in qkv_steps(g):
            fn(arg)
    for g in range(2, NG):
        fillers.extend(qkv_steps(g))

    # ---------------- attention ------------------------------------------
    blocks = [(b, qi) for b in range(B) for qi in range(NQB)]
    waves_left = [sum(qi + 1 for _, qi in blocks)]

    def emit_scores(b, qi, w, sp, last):
        # h-major slot order: consecutive matmuls fill PSUM banks
        # back-to-back (A,A,B,B) - the interleaved order aborts on hw
        qcols = slice(b * S + qi * 256, b * S + qi * 256 + 256)
        for h in range(HPC):
            for jj, j in enumerate((2 * w, 2 * w + 1)):
                kcols = slice(b * S + j * 128, b * S + j * 128 + 128)
                nc.tensor.matmul(sp[:, h, jj, :],
                                 kT[h * 64:(h + 1) * 64, kcols],
                                 qT[h * 64:(h + 1) * 64, qcols],
                                 start=True, stop=True)

    def emit_exp(sp, e, last):
        nc.scalar.activation(e[:], sp[:], EXPFN, scale=0.125)
        if last:
            # causal diagonal masks (both heads per op)
            nc.vector.tensor_mul(e[:, :, 0, 0:128], e[:, :, 0, 0:128],
                                 tri_s[:])
            nc.vector.tensor_mul(e[:, :, 1, 128:256], e[:, :, 1, 128:256],
                                 tri_s[:])

    def emit_av(b, qi, w, e, cts, nch):
        for h in range(HPC):
            for jj, j in enumerate((2 * w, 2 * w + 1)):
                lhsT = vaug[:, b * (S // 128) + j, h, :]
                first = (j == 0)
                last = (j == nch - 1)
                if last:  # odd diagonal: only the second 128 queries
                    nc.tensor.matmul(cts[h][:, 128:256], lhsT,
                                     e[:, h, 1, 128:256],
                                     start=first, stop=True)
                else:
                    nc.tensor.matmul(cts[h][:, :], lhsT, e[:, h, jj, :],
                                     start=first, stop=last)

    NO_FILL = os.environ.get("KERNEL_NO_FILL") == "1"
    for b, qi in blocks:
        # this block's scores/AV need QKV groups through 4b + qi//2 emitted
        drain_qkv_through(4 * b + qi // 2)
        nch = 2 * (qi + 1)
        nwaves = nch // 2
        qcols = slice(b * S + qi * 256, b * S + qi * 256 + 256)
        # separate tiles: each accumulation group needs its own PSUM bank
        # (2 KB zero region) since both heads' groups stay open at once
        ct0 = psC.tile([65, 256], F32, tag="ct0")
        ct1 = psC.tile([65, 256], F32, tag="ct1")
        cts = [ct0, ct1]
        prev = None
        for w in range(nwaves):
            last = (w == nwaves - 1)
            sp = psS.tile([128, HPC, 2, 256], F32, tag="sp")
            emit_scores(b, qi, w, sp, last)
            e = ep.tile([128, HPC, 2, 256], MM_DT, tag="e")
            emit_exp(sp, e, last)
            if prev is not None:
                emit_av(b, qi, prev[0], prev[1], cts, nch)
            prev = (w, e)
            # pace fillers so the PE never starves while ACT runs exp
            waves_left[0] -= 1
            if not NO_FILL:
                remaining = len(fillers) - fpos[0]
                if remaining > 0:
                    per = -(-remaining // max(waves_left[0], 1))
                    emit_filler(min(per, 3))
        emit_av(b, qi, prev[0], prev[1], cts, nch)

        # normalize: ctxT[:, q] = ct[0:64] * (1 / ct[64]) broadcast
        for h in range(HPC):
            # reciprocal_approx_fast reading PSUM directly returns garbage
            # on hardware - stage the denominators through SBUF first
            r = rp.tile([1, 256], F32, tag="r")
            sums = rp.tile([1, 256], F32, tag="r", name="sums")
            nc.vector.tensor_copy(sums[:], cts[h][64:65, :])
            nc.vector.reciprocal_approx_fast(r[:], sums[:])
            rb = rbp.tile([64, 256], F32, tag="rb")
            nc.gpsimd.partition_broadcast(rb[:], r[:])
            nc.vector.tensor_mul(ctxT[h * 64:(h + 1) * 64, qcols],
                                 cts[h][0:64, :], rb[:])

        if qi % 2 == 1:
            fillers.extend(outproj_steps(b * (S // 512) + qi // 2))
        if NO_FILL:
            emit_filler(len(fillers))

    emit_filler(len(fillers))

    if t_dbg is not None:
        for src, dst in zip((qT, kT, vT, ctxT), t_dbg[:4]):
            nc.sync.dma_start(dst[:], src[:])
        nc.sync.dma_start(t_dbg[4][:], vaug[:])


_NC = None


def _build():
    global _NC
    if _NC is not None:
        return _NC
    nc = bacc.Bacc("TRN2", target_bir_lowering=False, debug=False,
                   num_devices=NCORES)
    t_in = [
        nc.dram_tensor("xt", [NG, 128, KC, 512], MM_DT, kind="ExternalInput").ap(),
        nc.dram_tensor("wq", [128, KC, 128], MM_DT, kind="ExternalInput").ap(),
        nc.dram_tensor("wk", [128, KC, 128], MM_DT, kind="ExternalInput").ap(),
        nc.dram_tensor("wv", [128, KC, 128], MM_DT, kind="ExternalInput").ap(),
        nc.dram_tensor("wo", [128, D], MM_DT, kind="ExternalInput").ap(),
        nc.dram_tensor("bq", [128, 1], F32, kind="ExternalInput").ap(),
        nc.dram_tensor("bk", [128, 1], F32, kind="ExternalInput").ap(),
        nc.dram_tensor("tri2", [128, 2, 128], MM_DT, kind="ExternalInput").ap(),
        nc.dram_tensor("ident", [128, 128], MM_DT, kind="ExternalInput").ap(),
        nc.dram_tensor("vones", [128, NKV * HPC], MM_DT,
                       kind="ExternalInput").ap(),
    ]
    po = nc.dram_tensor("po", [KC, 128, T], MM_DT, kind="ExternalOutput").ap()
    t_dbg = None
    if os.environ.get("KERNEL_DEBUG_TAPS") == "1":
        t_dbg = [
            nc.dram_tensor("dbg_qT", [128, T], MM_DT, kind="ExternalOutput").ap(),
            nc.dram_tensor("dbg_kT", [128, T], MM_DT, kind="ExternalOutput").ap(),
            nc.dram_tensor("dbg_vT", [128, T], MM_DT, kind="ExternalOutput").ap(),
            nc.dram_tensor("dbg_ctxT", [128, T], MM_DT, kind="ExternalOutput").ap(),
            nc.dram_tensor("dbg_vaug", [128, NKV, HPC, 65], MM_DT,
                           kind="ExternalOutput").ap(),
        ]
    with tile.TileContext(nc) as tc, ExitStack() as ctx:
        _body(nc, tc, ctx, t_in, po, t_dbg)
    nc.compile()
    _NC = nc
    return nc


def _in_maps(hidden_states, Wq, bq, Wk, bk, Wv, bv, Wo, bo):
    hid = np.asarray(hidden_states, dtype=np.float32).reshape(T, D)
    hidT = hid.T.astype(MM_NP)                       # [D, T]
    xt = np.ascontiguousarray(
        hidT.reshape(KC, 128, NG, 512).transpose(2, 1, 0, 3))
    tri = np.triu(np.ones((128, 128), MM_NP))
    common = {
        "xt": xt,
        "tri2": np.ascontiguousarray(
            np.broadcast_to(tri[:, None, :], (128, 2, 128))),
        "ident": np.eye(128, dtype=MM_NP),
        "vones": np.ones((128, NKV * HPC), MM_NP),
    }
    maps = []
    for c in range(NCORES):
        cs = slice(c * 128, (c + 1) * 128)
        maps.append(dict(
            common,
            wq=np.ascontiguousarray(np.asarray(Wq)[:, cs].astype(MM_NP).reshape(KC, 128, 128).transpose(1, 0, 2)),
            wk=np.ascontiguousarray(np.asarray(Wk)[:, cs].astype(MM_NP).reshape(KC, 128, 128).transpose(1, 0, 2)),
            wv=np.ascontiguousarray(np.asarray(Wv)[:, cs].astype(MM_NP).reshape(KC, 128, 128).transpose(1, 0, 2)),
            wo=np.ascontiguousarray(np.asarray(Wo)[cs, :].astype(MM_NP)),
            bq=np.asarray(bq)[cs].reshape(128, 1).astype(np.float32),
            bk=np.asarray(bk)[cs].reshape(128, 1).astype(np.float32),
        ))
    return maps


def kernel(hidden_states, Wq, bq, Wk, bk, Wv, bv, Wo, bo):
    nc = _build()
    maps = _in_maps(hidden_states, Wq, bq, Wk, bk, Wv, bv, Wo, bo)
    res = run_bass_kernel_spmd(nc, maps, list(range(NCORES))).results
    acc = np.zeros((KC, 128, T), np.float64)
    for r in res:
        acc += r["po"]
    outT = acc.reshape(D, T)
    out = outT.T
    # v-bias commutes through softmax (weights sum to 1); fold it, plus the
    # output bias, into the host-side reduction
    bias = np.asarray(bo, np.float64) + (
        np.asarray(bv, np.float64) @ np.asarray(Wo, np.float64))
    out = out + bias[None, :]
    return out.reshape(B, S, D).astype(np.float32)


# revision 43
# speedup vs baseline: 1.3167x; 1.0131x over previous
"""Multi-head causal attention (B=2, S=2048, H=16, DH=64, D=1024) on 8 TRN2 cores.

Sharding: Megatron tensor-parallel over heads - core c owns heads {2c, 2c+1}:
  * column-slices of Wq/Wk/Wv (128 cols each) + bias slices,
  * row-slice of Wo (128 rows),
  * full hidden_states (pre-transposed on host to [D, B*S]).
Each core computes a partial output (its 2 heads through Wo rows); host sums
the 8 partials and adds bo + bv@Wo (the v-bias commutes through softmax-
weighted averaging and the output projection).

Device dataflow per core (fp16 matmuls):
  A) QKV: qT/kT/vT [128, 4096] = W_slice.T @ hiddenT (contraction over D in
     8 chunks of 128; q/k biases are per-partition adds during PSUM
     evacuation, v bias is folded into the host-side output bias).
     vT is DMA-transposed (XBAR) into V_aug [tok128, chunk, head, 65] whose
     65th column is set to 1 (memset) so the AV matmul also emits softmax
     denominators.
  B) Attention per (batch b, 256-query block qi), both heads, waves of 2 kv
     chunks: scoresT[kv,q] = kT_h.T @ qT_h (K=64, N=256), exp via ACT
     (scale=1/8) straight off PSUM, causal diagonal handled by a triangular
     multiply (even chunk) / restricting scores+AV to the second 128 queries
     (odd chunk), then ctxT_aug[65, q] += V_aug.T @ expT. Denominators
     (row 64) are reciprocal-ed off PSUM and partition-broadcast (gpsimd),
     then ctxT normalized via DVE.
  C) Output proj: outT[n,tok] = Wo_slice_chunk.T @ ctxT, DMA-ed to DRAM
     straight from PSUM (f32), no bias on device.

The emission order software-pipelines everything: attention waves for block
w are interleaved with the previous wave's AV matmuls, and "filler" PE work
(QKV for later token groups, output projections of finished 512-token
blocks) is slotted between waves so the tensor engine never goes idle (idle
gaps reset the PE to its mid p-state: 1.2 GHz instead of 2.4 GHz).
"""
import os
import sys

sys.path.insert(0, "/opt/trn_rl_repo")

from contextlib import ExitStack

import numpy as np

import concourse.bass as bass
import concourse.mybir as mybir
import concourse.tile as tile
from concourse import bacc
from concourse.bass_utils import run_bass_kernel_spmd

F32 = mybir.dt.float32
F16 = mybir.dt.float16
MM_DT = F16
MM_NP = np.float16

B, S, H, DH = 2, 2048, 16, 64
D = H * DH            # 1024
T = B * S             # 4096 tokens
NCORES = 8
HPC = H // NCORES     # 2 heads per core
KC = D // 128         # 8 contraction chunks
NG = T // 512         # 8 token groups for QKV
NQB = S // 256        # 8 query blocks per batch
NKV = T // 128        # 32 kv chunks of 128 tokens
EXPFN = mybir.ActivationFunctionType.Exp


def _body(nc, tc, ctx, t_in, t_out, t_dbg=None):
    xt, wq, wk, wv, wo, bq, bk, tri2, ident, vones = t_in
    po = t_out

    const = ctx.enter_context(tc.tile_pool(name="const", bufs=1))
    big = ctx.enter_context(tc.tile_pool(name="big", bufs=1))
    ep = ctx.enter_context(tc.tile_pool(name="ep", bufs=4))
    rp = ctx.enter_context(tc.tile_pool(name="rp", bufs=8))
    rbp = ctx.enter_context(tc.tile_pool(name="rbp", bufs=4))

    psQ = ctx.enter_context(tc.tile_pool(name="psQ", bufs=2, space="PSUM"))
    psS = ctx.enter_context(tc.tile_pool(name="psS", bufs=2, space="PSUM"))
    psC = ctx.enter_context(tc.tile_pool(name="psC", bufs=1, space="PSUM"))

    # ---- constants / weights in SBUF (gpsimd ring, parallel with xt on sync)
    wq_s = const.tile([128, KC, 128], MM_DT, tag="wq")
    wk_s = const.tile([128, KC, 128], MM_DT, tag="wk")
    wv_s = const.tile([128, KC, 128], MM_DT, tag="wv")
    wo_s = const.tile([128, D], MM_DT, tag="wo")
    bq_s = const.tile([128, 1], F32, tag="bq")
    bk_s = const.tile([128, 1], F32, tag="bk")
    tri_s = const.tile([128, 2, 128], MM_DT, tag="tri")
    id_s = const.tile([128, 128], MM_DT, tag="ident")
    qT = big.tile([128, T], MM_DT, tag="qT")
    kT = big.tile([128, T], MM_DT, tag="kT")
    vT = big.tile([128, T], MM_DT, tag="vT")
    ctxT = big.tile([128, T], MM_DT, tag="ctxT")
    vaug = big.tile([128, NKV, HPC, 65], MM_DT, tag="vaug")
    xall = big.tile([128, NG, KC, 512], MM_DT, tag="xall")

    # everything on the sync HWDGE ring, in consumption order (the gpsimd
    # software-DGE ring is too slow/loosely-synchronized for constants that
    # the now-early attention blocks consume within the first ~10us)
    nc.sync.dma_start(xall[:, 0], xt[0])
    nc.sync.dma_start(wq_s[:], wq[:])
    nc.sync.dma_start(wk_s[:], wk[:])
    nc.sync.dma_start(wv_s[:], wv[:])
    nc.sync.dma_start(bq_s[:], bq[:])
    nc.sync.dma_start(bk_s[:], bk[:])
    # ones column of V_aug (softmax denominator trick)
    nc.sync.dma_start(vaug[:, :, :, 64].rearrange("p c h -> p (c h)"),
                      vones[:])
    nc.sync.dma_start(tri_s[:], tri2[:])
    nc.sync.dma_start(id_s[:], ident[:])
    nc.sync.dma_start(wo_s[:], wo[:])
    for g in range(1, NG):
        nc.sync.dma_start(xall[:, g], xt[g])

    woc = wo_s[:].rearrange("p (c n) -> p c n", c=KC)

    # ---------------- QKV projection emission (per token group) ----------
    def qkv_steps(g):
        """Yield filler steps (2 matmuls each) for token group g."""
        cols = slice(g * 512, (g + 1) * 512)
        for w_s, b_s, dst in ((wq_s, bq_s, qT), (wk_s, bk_s, kT),
                              (wv_s, None, vT)):
            # allocate the PSUM accumulator lazily (at emission time) so the
            # pool's WAR dependencies always point backward in the PE stream
            box = [None]

            def mk(k0, box=box, w_s=w_s, b_s=b_s, dst=dst, g=g, cols=cols):
                if k0 == 0:
                    box[0] = psQ.tile([128, 512], F32, tag="acc",
                                      name="acc")
                acc = box[0]
                for k in (k0, k0 + 1):
                    nc.tensor.matmul(acc[:], w_s[:, k, :], xall[:, g, k, :],
                                     start=(k == 0), stop=(k == KC - 1))
                if k0 + 2 == KC:
                    # evacuate PSUM (+bias for q/k) on the vector engine
                    if b_s is not None:
                        nc.vector.tensor_scalar_add(dst[:, cols], acc[:],
                                                    b_s[:])
                    else:
                        nc.vector.tensor_copy(dst[:, cols], acc[:])
                        # transpose v into V_aug via the PE
                        for c in range(4):
                            chunk = g * 4 + c
                            tp = psQ.tile([128, 128], MM_DT, tag="acc",
                                          name="tp")
                            nc.tensor.transpose(
                                tp[:], vT[:, chunk * 128:(chunk + 1) * 128],
                                id_s[:])
                            nc.vector.tensor_copy(
                                vaug[:, chunk, :, 0:64],
                                tp[:].rearrange("p (h d) -> p h d", h=HPC))

            for k0 in range(0, KC, 2):
                yield ("qkv", g, mk, k0)

    # ---------------- output projection emission (per 512-token block) ---
    osp = ctx.enter_context(tc.tile_pool(name="osp", bufs=4))
    evac_rr = [0]

    def outproj_steps(t):
        tcols = slice(t * 512, (t + 1) * 512)

        def mk(c, tcols=tcols, t=t):
            op = psQ.tile([128, 512], F32, tag="acc")
            nc.tensor.matmul(op[:], woc[:, c, :], ctxT[:, tcols],
                             start=True, stop=True)
            ost = osp.tile([128, 512], MM_DT, tag="ost")
            # alternate the PSUM evacuation between vector and scalar
            # (gpsimd cannot read PSUM)
            i = evac_rr[0] % 2
            evac_rr[0] += 1
            if i == 0:
                nc.vector.tensor_copy(ost[:], op[:])
            else:
                nc.scalar.activation(ost[:], op[:],
                                     mybir.ActivationFunctionType.Copy)
            nc.sync.dma_start(po[c, :, tcols], ost[:])

        for c in range(KC):
            yield ("op", t, mk, c)

    # ---------------- filler machinery ----------------------------------
    fillers = []          # list of (tag, ident, fn, arg)
    fpos = [0]

    def emit_filler(n):
        end = min(fpos[0] + n, len(fillers))
        for i in range(fpos[0], end):
            tag, ident, fn, arg = fillers[i]
            fn(arg)
        fpos[0] = end

    def drain_qkv_through(g):
        """Force-emit all queued QKV steps for groups <= g."""
        i = fpos[0]
        while i < len(fillers) and fillers[i][0] == "qkv" \
                and fillers[i][1] <= g:
            tag, ident, fn, arg = fillers[i]
            fn(arg)
            i += 1
        fpos[0] = i

    # groups 0,1 emitted up front (needed before any attention); the rest
    # become fillers
    for g in (0, 1):
        for _, _, fn, arg in qkv_steps(g):
            fn(arg)
    for g in range(2, NG):
        fillers.extend(qkv_steps(g))

    # ---------------- attention ------------------------------------------
    blocks = [(b, qi) for b in range(B) for qi in range(NQB)]
    waves_left = [sum(qi + 1 for _, qi in blocks)]

    def emit_scores(b, qi, w, sp, last):
        # h-major slot order: consecutive matmuls fill PSUM banks
        # back-to-back (A,A,B,B) - the interleaved order aborts on hw
        qcols = slice(b * S + qi * 256, b * S + qi * 256 + 256)
        for h in range(HPC):
            for jj, j in enumerate((2 * w, 2 * w + 1)):
                kcols = slice(b * S + j * 128, b * S + j * 128 + 128)
                nc.tensor.matmul(sp[:, h, jj, :],
                                 kT[h * 64:(h + 1) * 64, kcols],
                                 qT[h * 64:(h + 1) * 64, qcols],
                                 start=True, stop=True)

    def emit_exp(sp, e, last):
        nc.scalar.activation(e[:], sp[:], EXPFN, scale=0.125)
        if last:
            # causal diagonal masks (both heads per op)
            nc.vector.tensor_mul(e[:, :, 0, 0:128], e[:, :, 0, 0:128],
                                 tri_s[:])
            nc.vector.tensor_mul(e[:, :, 1, 128:256], e[:, :, 1, 128:256],
                                 tri_s[:])

    def emit_av(b, qi, w, e, cts, nch):
        for h in range(HPC):
            for jj, j in enumerate((2 * w, 2 * w + 1)):
                lhsT = vaug[:, b * (S // 128) + j, h, :]
                first = (j == 0)
                last = (j == nch - 1)
                if last:  # odd diagonal: only the second 128 queries
                    nc.tensor.matmul(cts[h][:, 128:256], lhsT,
                                     e[:, h, 1, 128:256],
                                     start=first, stop=True)
                else:
                    nc.tensor.matmul(cts[h][:, :], lhsT, e[:, h, jj, :],
                                     start=first, stop=last)

    NO_FILL = os.environ.get("KERNEL_NO_FILL") == "1"
    for b, qi in blocks:
        # this block's scores/AV need QKV groups through 4b + qi//2 emitted
        drain_qkv_through(4 * b + qi // 2)
        nch = 2 * (qi + 1)
        nwaves = nch // 2
        qcols = slice(b * S + qi * 256, b * S + qi * 256 + 256)
        # separate tiles: each accumulation group needs its own PSUM bank
        # (2 KB zero region) since both heads' groups stay open at once
        ct0 = psC.tile([65, 256], F32, tag="ct0")
        ct1 = psC.tile([65, 256], F32, tag="ct1")
        cts = [ct0, ct1]
        prev = None
        for w in range(nwaves):
            last = (w == nwaves - 1)
            sp = psS.tile([128, HPC, 2, 256], F32, tag="sp")
            emit_scores(b, qi, w, sp, last)
            e = ep.tile([128, HPC, 2, 256], MM_DT, tag="e")
            emit_exp(sp, e, last)
            if prev is not None:
                emit_av(b, qi, prev[0], prev[1], cts, nch)
            prev = (w, e)
            # pace fillers so the PE never starves while ACT runs exp
            waves_left[0] -= 1
            if not NO_FILL:
                remaining = len(fillers) - fpos[0]
                if remaining > 0:
                    per = -(-remaining // max(waves_left[0], 1))
                    emit_filler(min(per, 3))
        emit_av(b, qi, prev[0], prev[1], cts, nch)

        # normalize: ctxT[:, q] = ct[0:64] * (1 / ct[64]) broadcast
        # stage raw ctx+denominator in SBUF first: a [65,256] DVE copy costs
        # the same as [1,256] (free-size bound), frees the ct PSUM bank after
        # one op instead of holding it through the whole reciprocal chain,
        # and reciprocal_approx_fast can't read PSUM on hw anyway
        # stage raw ctx+denominator in SBUF first: a [65,256] DVE copy costs
        # the same as [1,256] (free-size bound) and frees the ct PSUM bank
        # after one op instead of holding it through the reciprocal chain
        # ([65,256] single-copy variant NaNs on hw - 65-partition PSUM reads
        # are unreliable; split into 64+1)
        craws = []
        for h in range(HPC):
            craw = rp.tile([64, 256], F32, tag="r", name="craw")
            nc.vector.tensor_copy(craw[:], cts[h][0:64, :])
            sums = rp.tile([1, 256], F32, tag="r", name="sums")
            nc.vector.tensor_copy(sums[:], cts[h][64:65, :])
            craws.append((craw, sums))
        for h in range(HPC):
            craw, sums = craws[h]
            r = rp.tile([1, 256], F32, tag="r")
            nc.vector.reciprocal_approx_fast(r[:], sums[:])
            rb = rbp.tile([64, 256], F32, tag="rb")
            nc.gpsimd.partition_broadcast(rb[:], r[:])
            nc.vector.tensor_mul(ctxT[h * 64:(h + 1) * 64, qcols],
                                 craw[:], rb[:])

        if qi % 2 == 1:
            fillers.extend(outproj_steps(b * (S // 512) + qi // 2))
        if NO_FILL:
            emit_filler(len(fillers))

    emit_filler(len(fillers))

    if t_dbg is not None:
        for src, dst in zip((qT, kT, vT, ctxT), t_dbg[:4]):
            nc.sync.dma_start(dst[:], src[:])
        nc.sync.dma_start(t_dbg[4][:], vaug[:])


_NC = None


def _build():
    global _NC
    if _NC is not None:
        return _NC
    nc = bacc.Bacc("TRN2", target_bir_lowering=False, debug=False,
                   num_devices=NCORES)
    t_in = [
        nc.dram_tensor("xt", [NG, 128, KC, 512], MM_DT, kind="ExternalInput").ap(),
        nc.dram_tensor("wq", [128, KC, 128], MM_DT, kind="ExternalInput").ap(),
        nc.dram_tensor("wk", [128, KC, 128], MM_DT, kind="ExternalInput").ap(),
        nc.dram_tensor("wv", [128, KC, 128], MM_DT, kind="ExternalInput").ap(),
        nc.dram_tensor("wo", [128, D], MM_DT, kind="ExternalInput").ap(),
        nc.dram_tensor("bq", [128, 1], F32, kind="ExternalInput").ap(),
        nc.dram_tensor("bk", [128, 1], F32, kind="ExternalInput").ap(),
        nc.dram_tensor("tri2", [128, 2, 128], MM_DT, kind="ExternalInput").ap(),
        nc.dram_tensor("ident", [128, 128], MM_DT, kind="ExternalInput").ap(),
        nc.dram_tensor("vones", [128, NKV * HPC], MM_DT,
                       kind="ExternalInput").ap(),
    ]
    po = nc.dram_tensor("po", [KC, 128, T], MM_DT, kind="ExternalOutput").ap()
    t_dbg = None
    if os.environ.get("KERNEL_DEBUG_TAPS") == "1":
        t_dbg = [
            nc.dram_tensor("dbg_qT", [128, T], MM_DT, kind="ExternalOutput").ap(),
            nc.dram_tensor("dbg_kT", [128, T], MM_DT, kind="ExternalOutput").ap(),
            nc.dram_tensor("dbg_vT", [128, T], MM_DT, kind="ExternalOutput").ap(),
            nc.dram_tensor("dbg_ctxT", [128, T], MM_DT, kind="ExternalOutput").ap(),
            nc.dram_tensor("dbg_vaug", [128, NKV, HPC, 65], MM_DT,
                           kind="ExternalOutput").ap(),
        ]
    with tile.TileContext(nc) as tc, ExitStack() as ctx:
        _body(nc, tc, ctx, t_in, po, t_dbg)
    nc.compile()
    _NC = nc
    return nc


def _in_maps(hidden_states, Wq, bq, Wk, bk, Wv, bv, Wo, bo):
    hid = np.asarray(hidden_states, dtype=np.float32).reshape(T, D)
    hidT = hid.T.astype(MM_NP)                       # [D, T]
    xt = np.ascontiguousarray(
        hidT.reshape(KC, 128, NG, 512).transpose(2, 1, 0, 3))
    tri = np.triu(np.ones((128, 128), MM_NP))
    common = {
        "xt": xt,
        "tri2": np.ascontiguousarray(
            np.broadcast_to(tri[:, None, :], (128, 2, 128))),
        "ident": np.eye(128, dtype=MM_NP),
        "vones": np.ones((128, NKV * HPC), MM_NP),
    }
    maps = []
    for c in range(NCORES):
        cs = slice(c * 128, (c + 1) * 128)
        maps.append(dict(
            common,
            wq=np.ascontiguousarray(np.asarray(Wq)[:, cs].astype(MM_NP).reshape(KC, 128, 128).transpose(1, 0, 2)),
            wk=np.ascontiguousarray(np.asarray(Wk)[:, cs].astype(MM_NP).reshape(KC, 128, 128).transpose(1, 0, 2)),
            wv=np.ascontiguousarray(np.asarray(Wv)[:, cs].astype(MM_NP).reshape(KC, 128, 128).transpose(1, 0, 2)),
            wo=np.ascontiguousarray(np.asarray(Wo)[cs, :].astype(MM_NP)),
            bq=np.asarray(bq)[cs].reshape(128, 1).astype(np.float32),
            bk=np.asarray(bk)[cs].reshape(128, 1).astype(np.float32),
        ))
    return maps


def kernel(hidden_states, Wq, bq, Wk, bk, Wv, bv, Wo, bo):
    nc = _build()
    maps = _in_maps(hidden_states, Wq, bq, Wk, bk, Wv, bv, Wo, bo)
    res = run_bass_kernel_spmd(nc, maps, list(range(NCORES))).results
    acc = np.zeros((KC, 128, T), np.float64)
    for r in res:
        acc += r["po"]
    outT = acc.reshape(D, T)
    out = outT.T
    # v-bias commutes through softmax (weights sum to 1); fold it, plus the
    # output bias, into the host-side reduction
    bias = np.asarray(bo, np.float64) + (
        np.asarray(bv, np.float64) @ np.asarray(Wo, np.float64))
    out = out + bias[None, :]
    return out.reshape(B, S, D).astype(np.float32)
